# revision 3
# baseline (speedup 1.0000x reference)
"""Block-sparse attention kernel v2 for Trainium2 (8 NeuronCores, SPMD).

Design (v2) — engine-balanced flash attention in S^T layout:

* Shard batch*heads (32) across 8 cores, 4 heads each.
* Scores S^T[k,q] per 128-k-chunk via row-tiled fp32r matmul pairs
  (K=64 halves, free dim = 512-query blocks, trimmed to the sparsity).
* exp() is the true bottleneck (9.4M elements/core, ACT-only would be
  ~62us busy).  Split it across BOTH the scalar engine (exact Exp
  activation, bf16 out) and the vector engine (Schraudolph exp: one
  tensor_scalar mult+add with int16-converting store, bitcast to bf16;
  +-3% elementwise, cancels in softmax).  A host-side greedy balancer
  assigns whole chunk-groups to whichever engine has less load.
  GPSIMD cannot read PSUM, so it instead handles boundary masks,
  memsets and input-DMA triggers.
* PV in O[q,d] orientation: p (bf16) is the stationary operand
  [128k x 128q], V chunks (bf16, + ones column for the softmax
  denominators) stream as the moving operand -> full 128x128 array
  utilization, no output transposes.
* Normalization (divide by denominators), O^T bookkeeping and
  empty-row patching all happen on the host: the device ships raw
  [q, 66] tiles (64 values, denominator, pad) straight out of PSUM
  via one engine copy + DMA.
"""

import numpy as np
import ml_dtypes

import concourse.mybir as mybir
import concourse.tile as tile
from concourse import bacc
from concourse.bass_utils import run_bass_kernel_spmd

F32 = mybir.dt.float32
F32R = mybir.dt.float32r
F16 = mybir.dt.float16
BF16 = mybir.dt.bfloat16
I16 = mybir.dt.int16

B, H, N, D = 2, 16, 2048, 64
NCORES = 8
HPC = (B * H) // NCORES        # heads per core
CHUNK = 128                    # k-chunk
QP = 512                       # q-block
NQB = N // QP
NCHUNK = N // CHUNK
NSUB = QP // 128               # q-subtiles per block
DCOL = D + 2                   # V cols + denom ones + pad (even count)
GMAX = 2                       # chunks per exp group (2 psum banks)

LOG2E = 1.4426950408889634
# Schraudolph-16: p ~= bitcast_bf16(int16(s * A + B)).  B16_SHIFT is the
# minimax mantissa bias (in 1/128 exponent units), calibrated on device.
B16_BASE = 127.0 * 128.0
# int16 convert truncates (measured): fold +0.5 into the minimax bias
# sigma = 128*log2(sqrt((1+f)/2^f max)) = 5.58, minus 0.5 for truncation
B16_SHIFT = 5.08


def _runs(mask):
    idx = np.flatnonzero(np.diff(np.concatenate(([False], mask, [False])).astype(np.int8)))
    return list(zip(idx[0::2], idx[1::2]))


def _schedule(starts, ends):
    """Per q-block: chunk descriptors + equal-range groups + PV metadata."""
    qblocks = []
    for b in range(NQB):
        qb = b * QP
        ps = starts[qb:qb + QP]
        pe = ends[qb:qb + QP]
        chunks = []
        for c in range(NCHUNK):
            lo, hi = c * CHUNK, (c + 1) * CHUNK
            allowed = (pe > lo) & (ps < hi)
            if not allowed.any():
                continue
            nz = np.flatnonzero(allowed)
            qa, qz = int(nz[0]), int(nz[-1]) + 1
            la, lz = (qa // 128) * 128, ((qz + 127) // 128) * 128
            qa_e, qz_e = qa & ~1, min(QP, (qz + 1) & ~1)
            dis = _runs(~allowed)
            memsets = [(int(max(a, la)), int(min(z, lz)))
                       for a, z in dis if max(a, la) < min(z, lz)]
            mule = _runs(allowed & (pe > lo) & (pe < hi))
            muls = _runs(allowed & (ps > lo) & (ps < hi))
            chunks.append(dict(
                c=c, qa=qa_e, qz=qz_e, la=la, lz=lz,
                memsets=memsets,
                mule=[(int(a), int(z)) for a, z in mule],
                muls=[(int(a), int(z)) for a, z in muls],
            ))
        # group consecutive chunks with identical (la, lz), up to GMAX
        groups = []
        for ch in chunks:
            if (groups and len(groups[-1]) < GMAX
                    and groups[-1][0]["la"] == ch["la"]
                    and groups[-1][0]["lz"] == ch["lz"]):
                groups[-1].append(ch)
            else:
                groups.append([ch])
        if chunks:
            A = min(ch["qa"] for ch in chunks)
            Z = max(ch["qz"] for ch in chunks)
            nested = (chunks[0]["qa"] == A and chunks[0]["qz"] == Z)
        else:
            A = Z = 0
            nested = True
        qblocks.append(dict(groups=groups, A=A, Z=Z, nested=nested))
    return qblocks


class _Balance:
    """Greedy two-engine load balancer (ns estimates).

    exp groups carry a force hint: early q-blocks go to the exact ACT
    exp (few softmax terms -> Schraudolph noise wouldn't average out),
    late q-blocks to the DVE approximation (1000+ terms dilute the
    +-3% elementwise error ~sqrt(n)); mid blocks fill whichever engine
    is behind.
    """

    def __init__(self):
        self.load = {"act": 0.0, "dve": 0.0}

    def pick(self, act_cost, dve_cost, force=None):
        if force is None:
            force = ("act" if self.load["act"] + act_cost
                     <= self.load["dve"] + dve_cost else "dve")
        self.load[force] += act_cost if force == "act" else dve_cost
        return force


def _build(sched, sm_scale, use_me, use_ms, all_act=False):
    nc = bacc.Bacc("TRN2", target_bir_lowering=False, debug=True)

    kt_h = nc.declare_dram_parameter("kt", [HPC, 128, N], F16, isOutput=False)
    qt_h = nc.declare_dram_parameter("qt", [HPC, 128, N], F16, isOutput=False)
    vb_h = nc.declare_dram_parameter("vb", [HPC, 128, NCHUNK * DCOL], BF16,
                                     isOutput=False)
    me_h = ms_h = None
    if use_me:
        me_h = nc.declare_dram_parameter("me", [128, N], BF16, isOutput=False)
    if use_ms:
        ms_h = nc.declare_dram_parameter("ms", [128, N], BF16, isOutput=False)
    o_h = nc.declare_dram_parameter("o", [HPC, NQB, DCOL, QP], BF16,
                                    isOutput=True)

    exp_f = mybir.ActivationFunctionType.Exp
    copy_f = mybir.ActivationFunctionType.Copy
    MULT = mybir.AluOpType.mult
    ADD = mybir.AluOpType.add
    a16 = 128.0 * LOG2E * float(sm_scale)
    b16 = B16_BASE - B16_SHIFT

    bal = _Balance()

    with tile.TileContext(nc) as tc:
        with (
            tc.tile_pool(name="singles", bufs=1) as singles,
            tc.tile_pool(name="heads", bufs=4) as heads,
            tc.tile_pool(name="pbuf", bufs=6) as pbuf,
            tc.tile_pool(name="obuf", bufs=8) as obuf,
            tc.tile_pool(name="spsum", bufs=3, space="PSUM") as spsum,
            tc.tile_pool(name="opsum", bufs=2, space="PSUM") as opsum,
        ):
            head_sb = {}

            def load_head(g):
                kt_sb = heads.tile([128, N], F16, tag="kt", name=f"kt_{g}")
                qt_sb = heads.tile([128, N], F16, tag="qt", name=f"qt_{g}")
                vb_sb = heads.tile([128, NCHUNK * DCOL], BF16, tag="vb",
                                   name=f"vb_{g}")
                # head 0's qt rides the otherwise-idle sync ring (fast
                # startup); later heads' qt go on the gpsimd ring so the
                # sync ring stays clear for output DMAs (an out-DMA stuck
                # behind a 0.5MB qt load locks an obuf slot and cascades a
                # copy stall into the exp engine queues).
                nc.gpsimd.dma_start(out=kt_sb, in_=kt_h[g, :, :])
                if g == 0:
                    nc.sync.dma_start(out=qt_sb, in_=qt_h[g, :, :])
                else:
                    nc.gpsimd.dma_start(out=qt_sb, in_=qt_h[g, :, :])
                nc.gpsimd.dma_start(out=vb_sb, in_=vb_h[g, :, :])
                head_sb[g] = (kt_sb, qt_sb, vb_sb)

            load_head(0)
            me_sb = ms_sb = None
            if use_me:
                me_sb = singles.tile([128, N], BF16, tag="me")
                nc.gpsimd.dma_start(out=me_sb, in_=me_h[:, :])
            if use_ms:
                ms_sb = singles.tile([128, N], BF16, tag="ms")
                nc.gpsimd.dma_start(out=ms_sb, in_=ms_h[:, :])

            # flat item list: one item per (head, qblock, group).  Heads are
            # paired; the pair's two streams are zippered at GROUP
            # granularity with a 2-qblock phase offset, so head A's
            # ACT-assigned early q-blocks interleave 1:1 with head B's
            # DVE-assigned late q-blocks and both engines stay busy.
            import itertools
            items = []
            for g0 in range(0, HPC, 2):
                streams = []
                for g, border in ((g0, (0, 1, 2, 3)), (g0 + 1, (2, 3, 1, 0))):
                    st = []
                    for b in border:
                        blk = sched[b]
                        ngr = len(blk["groups"])
                        for k, grp in enumerate(blk["groups"]):
                            st.append(dict(g=g, b=b, grp=grp, blk=blk,
                                           blk_last=(k == ngr - 1)))
                    streams.append(st)
                items.extend(x for tup in itertools.zip_longest(*streams)
                             for x in tup if x is not None)

            mm_parity = 0

            def emit_pv(it, p_sb, first_c, last_c):
                """PV for one exp group: O^T[66, q] += vb_c^T @ p_c.

                One accumulation group per (head, qblock) psum bank; the
                group's matmuls interleave only with other banks' work.
                """
                vb_sb = head_sb[it["g"]][2]
                o_ps = it["o_ps"]
                for ci, ch in enumerate(it["grp"]):
                    c = ch["c"]
                    nc.tensor.matmul(
                        o_ps[:, ch["qa"]:ch["qz"]],
                        lhsT=vb_sb[:, c * DCOL:(c + 1) * DCOL],
                        rhs=p_sb[:, ci * QP + ch["qa"]:ci * QP + ch["qz"]],
                        start=(c == first_c and it["blk"]["nested"]),
                        stop=(c == last_c),
                    )
                if it["grp"][-1]["c"] == last_c:
                    g, b, blk = it["g"], it["b"], it["blk"]
                    A, Z = blk["A"], blk["Z"]
                    o_sb = obuf.tile([DCOL, QP], BF16, tag="osb",
                                     name=f"o_{g}_{b}")
                    w = Z - A
                    eng = bal.pick(w * 0.833 + 340, w * 1.042 + 250)
                    if eng == "act":
                        nc.scalar.activation(o_sb[:, A:Z], o_ps[:, A:Z],
                                             copy_f, scale=1.0)
                    else:
                        nc.vector.tensor_copy(o_sb[:, A:Z], o_ps[:, A:Z])
                    nc.sync.dma_start(out=o_h[g, b, :, A:Z],
                                      in_=o_sb[:, A:Z])

            pending = []
            PV_LAG = 2
            o_tiles = {}

            nload = len(items) // 5
            for i, it in enumerate(items):
                g, b, blk, grp = it["g"], it["b"], it["blk"], it["grp"]
                if i == 0 and 1 not in head_sb and HPC > 1:
                    load_head(1)
                if i == nload:
                    for g2 in range(2, HPC):
                        if g2 not in head_sb:
                            load_head(g2)

                kt_sb, qt_sb, vb_sb = head_sb[g]
                if (g, b) not in o_tiles:
                    # first group of a new qblock: one-bank psum accumulator
                    o_ps = opsum.tile([DCOL, QP], F32, tag="o",
                                      name=f"ops_{g}_{b}")
                    if not blk["nested"]:
                        nc.vector.memset(o_ps[:, blk["A"]:blk["Z"]], 0)
                    o_tiles[(g, b)] = o_ps
                o_ps = o_tiles[(g, b)]
                it["o_ps"] = o_ps
                it["first_c"] = blk["groups"][0][0]["c"]
                it["last_c"] = blk["groups"][-1][-1]["c"]

                ng = len(grp)
                s_t = spsum.tile([128, GMAX * QP], F32, tag="s",
                                 name=f"s_{i}")
                qb = b * QP
                # QK matmuls (row-tiled pairs via alternating halves)
                for ci, ch in enumerate(grp):
                    c = ch["c"]
                    pp = 64 * (mm_parity % 2)
                    mm_parity += 1
                    nc.tensor.matmul(
                        s_t[:, ci * QP + ch["qa"]:ci * QP + ch["qz"]],
                        lhsT=kt_sb[pp:pp + 64, c * CHUNK:(c + 1) * CHUNK],
                        rhs=qt_sb[pp:pp + 64, qb + ch["qa"]:qb + ch["qz"]],
                        start=True, stop=True,
                        tile_position=(pp, 0),
                    )

                # exp on ACT (exact) or DVE (Schraudolph), whole group
                la, lz = grp[0]["la"], grp[0]["lz"]
                w = lz - la
                p_sb = pbuf.tile([128, GMAX * QP], BF16, tag="p",
                                 name=f"p_{i}")
                s_view = (s_t[:, :ng * QP]
                          .rearrange("p (c q) -> p c q", q=QP)[:, :, la:lz])
                p_view = (p_sb[:, :ng * QP]
                          .rearrange("p (c q) -> p c q", q=QP)[:, :, la:lz])
                if all_act:
                    eng = bal.pick(ng * w * 0.833 + 340, 0, force="act")
                else:
                    force = "act" if b <= 1 else ("dve" if b == 3 else None)
                    eng = bal.pick(ng * w * 0.833 + 340,
                                   ng * w * 1.042 + 250, force=force)
                if eng == "act":
                    nc.scalar.activation(p_view, s_view, exp_f,
                                         scale=float(sm_scale))
                else:
                    nc.vector.tensor_scalar(p_view.bitcast(I16), s_view,
                                            a16, b16, MULT, ADD)

                # masks + memsets on GPSIMD (SBUF-only engine)
                for ci, ch in enumerate(grp):
                    for a, z in ch["memsets"]:
                        nc.gpsimd.memset(p_sb[:, ci * QP + a:ci * QP + z], 0)
                    for a, z in ch["mule"]:
                        nc.gpsimd.tensor_mul(
                            p_sb[:, ci * QP + a:ci * QP + z],
                            p_sb[:, ci * QP + a:ci * QP + z],
                            me_sb[:, qb + a:qb + z])
                    for a, z in ch["muls"]:
                        nc.gpsimd.tensor_mul(
                            p_sb[:, ci * QP + a:ci * QP + z],
                            p_sb[:, ci * QP + a:ci * QP + z],
                            ms_sb[:, qb + a:qb + z])

                pending.append((it, p_sb))
                if len(pending) > PV_LAG:
                    pit, pp = pending.pop(0)
                    emit_pv(pit, pp, pit["first_c"], pit["last_c"])
            for pit, pp in pending:
                emit_pv(pit, pp, pit["first_c"], pit["last_c"])

    print(f"[v2 build] est engine load ns: act={bal.load['act']:.0f} "
          f"dve={bal.load['dve']:.0f}")
    nc.compile()
    return nc


_CACHE = {}


def _get_program(starts, ends, sm_scale, use_me, use_ms):
    key = (starts.tobytes(), ends.tobytes(), float(sm_scale), use_me, use_ms)
    if key not in _CACHE:
        sched = _schedule(starts, ends)
        _CACHE[key] = (_build(sched, float(sm_scale), use_me, use_ms), sched)
    return _CACHE[key]


def _prep_inputs(q, k, v, starts, ends, use_me, use_ms):
    qf = np.asarray(q, np.float32).reshape(B * H, N, D)
    kf = np.asarray(k, np.float32).reshape(B * H, N, D)
    vf = np.asarray(v, np.float32).reshape(B * H, N, D)

    rows = np.arange(128, dtype=np.int64)[:, None]
    me = (rows < (ends[None, :] % CHUNK)).astype(ml_dtypes.bfloat16)
    ms = (rows >= (starts[None, :] % CHUNK)).astype(ml_dtypes.bfloat16)

    in_maps = []
    for i in range(NCORES):
        sl = slice(i * HPC, (i + 1) * HPC)
        kt1 = kf[sl].transpose(0, 2, 1)
        qt1 = qf[sl].transpose(0, 2, 1)
        kt = np.ascontiguousarray(
            np.concatenate([kt1, kt1], axis=1)).astype(np.float16)
        qt = np.ascontiguousarray(
            np.concatenate([qt1, qt1], axis=1)).astype(np.float16)
        vb = np.zeros([HPC, 128, NCHUNK, DCOL], np.float32)
        vb[:, :, :, :D] = vf[sl].reshape(HPC, NCHUNK, CHUNK, D).transpose(0, 2, 1, 3)
        vb[:, :, :, D] = 1.0
        vb = np.ascontiguousarray(
            vb.reshape(HPC, 128, NCHUNK * DCOL)).astype(ml_dtypes.bfloat16)
        m = {"kt": kt, "qt": qt, "vb": vb}
        if use_me:
            m["me"] = me
        if use_ms:
            m["ms"] = ms
        in_maps.append(m)
    return in_maps


def _run(inputs, trace=False):
    q, k, v = inputs["q"], inputs["k"], inputs["v"]
    sm_scale = float(np.asarray(inputs["sm_scale"]))
    starts = np.clip(np.asarray(inputs["row_starts"], np.int64), 0, N)
    ends = np.clip(np.asarray(inputs["row_ends"], np.int64), 0, N)

    use_ms = bool((starts % CHUNK).any())
    use_me = bool(((ends % CHUNK) * (ends > starts)).any())

    nc, sched = _get_program(starts, ends, sm_scale, use_me, use_ms)
    in_maps = _prep_inputs(q, k, v, starts, ends, use_me, use_ms)
    res = run_bass_kernel_spmd(nc, in_maps, list(range(NCORES)), trace=trace)

    raw = np.empty([B * H, NQB, DCOL, QP], np.float32)
    for i in range(NCORES):
        raw[i * HPC:(i + 1) * HPC] = res.results[i]["o"].astype(np.float32)
    raw = raw.transpose(0, 1, 3, 2).reshape(B, H, N, DCOL)
    vals = raw[..., :D]
    den = raw[..., D:D + 1]
    with np.errstate(divide="ignore", invalid="ignore"):
        out = vals / den
    empty = ends <= starts
    if empty.any():
        mean_v = np.asarray(v, np.float32).mean(axis=2)
        out[:, :, empty, :] = mean_v[:, :, None, :]
    return np.ascontiguousarray(out.astype(np.float32)), res.exec_time_ns


def kernel(**inputs) -> np.ndarray:
    out, _ = _run(inputs, trace=False)
    return out
